# revision 1
# baseline (speedup 1.0000x reference)
"""Trainium2 Bass kernel for nn_Network_79061757985000 (dense_mlp).

  h = x @ binarize(W1).T          [65536, 300]
  h = batchnorm(h, gamma1, beta1)
  o = h @ binarize(W2).T          [65536, 10]
  out = batchnorm(o, gamma2, beta2)

Strategy (8 NeuronCores, pure data parallelism over the batch):
  - Each core handles 8192 rows of x, cast fp32->fp16 during the
    HBM->SBUF DMA (SWDGE cast).  A row permutation (hT column 128*s + q
    holds input row 64*q + s) makes both the loads and the final store
    contiguous per partition, so each is ~128 DMA descriptors.
  - x tiles are transposed into [d, b] layout on the PE (pipelined
    transpose-mode matmuls); DMA xbar transposes serialize behind all
    outstanding DMA traffic, so they are only used for the last two
    chunks (after the cast stream has drained) and for the final
    [16, b] -> [b, 16] output transpose during the BN2 AllGather.
  - Layer 1: out[k_chunk<=128, 512] = W1bT[d,k].T @ xT[d, 512]
    (fp16 operands, fp32 PSUM accumulation, 7 K-chunks of <=128).
  - BN1 stats via DVE bn_stats on the PSUM tiles; per-core Welford
    triples are AllGather'd (4.6 KB) and re-aggregated locally.
  - BN1 + layer 2 are folded: o' = (h * a1) @ W2b.T with
    a1 = gamma1*rsqrt(var+eps); the remaining affine constants of BN1
    are batch-constant and cancel inside BN2.
  - Layer 2: out[10, 512] = W2aT[k,10].T @ hT[k, 512] (fp16), evacuated
    into a 16-partition-padded oT tile for the xbar output transpose.
  - BN2 stats are aggregated locally to one (count, mean, M2) triple per
    feature before a 120-byte AllGather; the final affine runs on the
    transposed [128, 64, 10] buffer with PE-broadcast a2/b2 rows.

The scale factors of the binarized matmuls cancel inside the batchnorms,
so fp16 inputs only contribute ~5e-4 relative error.
"""
import sys

sys.path.insert(0, "/opt/trn_rl_repo")

import numpy as np

import concourse.bass as bass
import concourse.tile as tile
from concourse import bacc, masks, mybir
from concourse import bass_utils

N_CORES = 8
B_FULL = 65536
BC = B_FULL // N_CORES          # 8192 rows per core
D = 784                         # input features
ND = 7                          # d-chunks of 128 (784 -> 896 padded)
DPAD = ND * 128                 # 896
H = 300                         # hidden features
KCH = [(0, 128), (128, 128), (256, 44)]   # (k0, kc) chunks of H
O = 10                          # output features
EPS = 1e-5
CAST_ROWS = 1024                # rows per cast-DMA chunk
NCHUNK = BC // CAST_ROWS        # 8
SLABS = CAST_ROWS // 128        # 8 slabs of 128 rows
GW = 512                        # moving free dim per matmul group
NGRP = BC // GW                 # 16 groups per core

f32 = mybir.dt.float32
f16 = mybir.dt.float16
AF = mybir.ActivationFunctionType
ALU = mybir.AluOpType


def ceil16(v):
    return (v + 15) // 16 * 16


def _emit(nc, tc, io, P, ranks, debug, l1_only=False):
    """Emit one full forward pass."""
    inv_n = 1.0 / (BC * ranks)
    pp, wtmp, xio, xTp, scr = P["pp"], P["wtmp"], P["xio"], P["xTp"], P["scr"]
    ps_h, ps_t, ps_w, dram = (P["ps_h"], P["ps_t"], P["ps_w"], P["dram"])
    ps_o = ps_h

    # ---------------- prefetch first x chunks ----------------
    # Row permutation: hT/oT column 128*s + q holds input row 64*q + s
    # (s = 8*c + g).  This makes both the HBM loads (25 KB contiguous per
    # partition per chunk -> 128 descriptors) and the final store (2.5 KB
    # contiguous per partition) descriptor-cheap.  BN stats are
    # permutation-invariant, so only the two HBM access patterns change.
    xsrc = io["x"].ap().rearrange("(q s) d -> q s d", q=128)

    # small weight/param loads first: they grab DMA sem lanes before the
    # big transfers, avoiding false serialization on recycled lanes
    w1f = wtmp.tile([128, 3, DPAD], f32, tag="w1f", name="w1f")
    nc.scalar.dma_start(
        w1f[:, 0:2, 0:D],
        io["W1"].ap()[0:256, :].rearrange("(c p) d -> p c d", p=128))
    nc.scalar.dma_start(w1f[0:44, 2:3, 0:D],
                        io["W1"].ap()[256:300, :].unsqueeze(1))
    w2f = wtmp.tile([O, H], f32, tag="w2f", name="w2f")
    nc.scalar.dma_start(w2f[:], io["W2"].ap())
    g1sb = pp.tile([128, 3], f32, tag="g1sb", name="g1sb")
    for ci, (k0, kc) in enumerate(KCH):
        nc.scalar.dma_start(g1sb[0:kc, ci:ci + 1],
                            io["gamma1"].ap()[k0:k0 + kc, :])
    # gamma2/beta2 staged as free-dim rows next to the (future) gathered
    # BN2 sums, so one ones-matmul broadcasts all of it to 128 partitions
    stage = pp.tile([1, 20 * ranks + 2 * O], f32, tag="stage", name="stage")
    nc.scalar.dma_start(stage[0:1, 20 * ranks:20 * ranks + O],
                        io["gamma2"].ap().rearrange("a b -> (a b)").unsqueeze(0))
    nc.scalar.dma_start(stage[0:1, 20 * ranks + O:20 * ranks + 2 * O],
                        io["beta2"].ap().rearrange("a b -> (a b)").unsqueeze(0))

    # w1s zero-fill on the vector queue: gpsimd is busy generating cast
    # descriptors and would gate the sign -> w1bT -> first-matmul chain
    w1s = wtmp.tile([128, 3, DPAD], f16, tag="w1s", name="w1s")
    nc.vector.memset(w1s[:, :, D:DPAD], 0.0)
    nc.vector.memset(w1s[:, 2, :], 0.0)

    x16_0 = xio.tile([128, SLABS, DPAD], f16, tag="x16", name="x16")
    nc.vector.memset(x16_0[:, :, D:DPAD], 0.0)
    for hh in range(4):
        hs = SLABS // 4
        nc.gpsimd.dma_start(
            x16_0[:, hs * hh:hs * (hh + 1), 0:D],
            xsrc[:, hs * hh:hs * (hh + 1), :])

    nc.scalar.sign(w1s[:, 0:2, 0:D], w1f[:, 0:2, 0:D])
    nc.scalar.sign(w1s[0:44, 2, 0:D], w1f[0:44, 2, 0:D])

    x16_1 = xio.tile([128, SLABS, DPAD], f16, tag="x16", name="x16")
    nc.vector.memset(x16_1[:, :, D:DPAD], 0.0)
    nc.gpsimd.dma_start(
        x16_1[:, :, 0:D], xsrc[:, SLABS:2 * SLABS, :])

    x16_2 = xio.tile([128, SLABS, DPAD], f16, tag="x16", name="x16")
    nc.vector.memset(x16_2[:, :, D:DPAD], 0.0)
    nc.gpsimd.dma_start(
        x16_2[:, :, 0:D], xsrc[:, 2 * SLABS:3 * SLABS, :])

    # ---------------- weight prep ----------------
    # w1bT via PE transposes (xbar DMA transposes serialize behind all
    # outstanding DMA traffic, so keep them out of the load stream)
    i10_16 = pp.tile([O, O], f16, tag="i10_16", name="i10_16")
    masks.make_identity(nc, i10_16[:])
    i128_16 = pp.tile([128, 128], f16, tag="i128_16", name="i128_16")
    masks.make_identity(nc, i128_16[:])

    w1bT = []
    for ci, (k0, kc) in enumerate(KCH):
        pc = ceil16(kc)
        wT = pp.tile([128, ND, pc], f16, tag=f"w1bT{ci}", name=f"w1bT{ci}")
        for j in range(ND):
            wps = ps_t.tile([128, pc], f16, tag="otps", name="wps")
            nc.tensor.transpose(wps[:],
                                w1s[0:pc, ci, 128 * j:128 * (j + 1)],
                                i128_16[0:pc, 0:pc])
            nc.vector.tensor_copy(wT[:, j, :], wps[:])
        w1bT.append(wT)

    w2s = wtmp.tile([O, H], f16, tag="w2s", name="w2s")
    nc.scalar.sign(w2s[:], w2f[:])
    w2bT = []
    for ci, (k0, kc) in enumerate(KCH):
        tps = ps_w.tile([128, O], f16, tag="wps", name="wps")
        nc.tensor.transpose(tps[0:kc, :], w2s[:, k0:k0 + kc], i10_16[:])
        wt = pp.tile([128, O], f16, tag=f"w2bT{ci}", name=f"w2bT{ci}")
        nc.vector.tensor_copy(wt[0:kc, :], tps[0:kc, :])
        w2bT.append(wt)

    # ---------------- persistent state ----------------
    hT = [pp.tile([128, BC], f16, tag=f"hT{ci}", name=f"hT{ci}")
          for ci in range(3)]
    bst = pp.tile([128, 3, NGRP, 6], f32, tag="bst", name="bst")
    # rows 10:16 stay uninitialized: their transposed image
    # outbuf16[:, :, 10:16] is never read
    oT16 = pp.tile([16, BC], f16, tag="oT16", name="oT16")
    bst2 = pp.tile([O, NGRP, 6], f32, tag="bst2", name="bst2")
    outbuf16 = pp.tile([128, BC // 128, 16], f16, tag="outbuf16",
                       name="outbuf16")
    outbuf32 = pp.tile([128, BC // 128, O], f32, tag="outbuf32",
                       name="outbuf32")

    # ---------------- layer 1 ----------------
    for c in range(NCHUNK):
        if c == 0:
            x16 = x16_0
        elif c == 1:
            x16 = x16_1
        elif c == 2:
            x16 = x16_2
        else:
            # slots recycle with period 4: chunk 3 gets the fresh 4th slot
            # (zero its pad once); chunks 4+ reuse pre-zeroed slots
            x16 = xio.tile([128, SLABS, DPAD], f16, tag="x16", name="x16")
            if c == 3:
                nc.vector.memset(x16[:, :, D:DPAD], 0.0)
            nc.gpsimd.dma_start(
                x16[:, :, 0:D], xsrc[:, c * SLABS:(c + 1) * SLABS, :])

        # transpose [128 b, 896 d] -> [128 d, 7 j, 128 b].  Early chunks go
        # through the PE (back-to-back PE transposes pipeline at ~N/f; xbar
        # DMA transposes serialize behind the outstanding cast-load stream).
        # Late chunks (casts finished by then, DMA stream idle) go through
        # the xbar to cut PE cycles, which also eases the PE power throttle.
        # Two dest tiles, one per matmul group, avoid WAW chains.
        xTt = []
        for half in range(2):
            # last two chunks (and the latest-needed half of chunk 5) via
            # xbar: by the time they are needed the cast stream (which DMA
            # transposes serialize behind) has drained; takes ~12us of
            # work off the power-throttled PE
            use_xbar = c >= NCHUNK - 2 or (c == NCHUNK - 3 and half == 1)
            xT2 = xTp.tile([128, 4, ND, 128], f16, tag=f"xT2{half}",
                           name=f"xT2{half}")
            for gg in range(4):
                g = 4 * half + gg
                if use_xbar:
                    nc.sync.dma_start(xT2[:, gg:gg + 1, :, :],
                                      x16[:, g:g + 1, :], transpose=True)
                    continue
                tpx = ps_t.tile([128, ND, 128], f16, tag="otps", name="tpx")
                for j in range(ND):
                    nc.tensor.transpose(
                        tpx[:, j, :], x16[:, g:g + 1, 128 * j:128 * (j + 1)],
                        i128_16[:])
                if g % 2 == 0:
                    nc.scalar.copy(xT2[:, gg, :, :], tpx[:])
                else:
                    nc.vector.tensor_copy(xT2[:, gg, :, :], tpx[:])
            xTt.append(xT2)

        for g2 in range(CAST_ROWS // GW):
            g = c * (CAST_ROWS // GW) + g2
            xT2 = xTt[g2]
            for ci, (k0, kc) in enumerate(KCH):
                hp = ps_h.tile([128, GW], f32, tag="hps", name="hps")
                for j in range(ND):
                    nc.tensor.matmul(
                        hp[0:kc, :],
                        w1bT[ci][:, j:j + 1, 0:kc],
                        xT2[:, :, j:j + 1, :],
                        start=(j == 0), stop=(j == ND - 1))
                # evacuate h to fp16 SBUF; batch stats via DVE bn_stats
                nc.scalar.copy(hT[ci][0:kc, GW * g:GW * (g + 1)], hp[0:kc, :])
                nc.vector.bn_stats(bst[0:kc, ci, g, :], hp[0:kc, :])

    if debug:
        for ci in range(3):
            nc.sync.dma_start(io["h_dbg"].ap()[ci:ci + 1, :, :], hT[ci][:])

    # ---------------- BN1 stats exchange ----------------
    # local aggregate per chunk, rebuild (count, mean, M2) triples, AllGather
    # triples built full-width in two ops; rows beyond kc of chunk 2 carry
    # garbage that the post-AG aggregation never reads
    locmv = pp.tile([128, 3, 2], f32, tag="locmv", name="locmv")
    trip = pp.tile([128, 3, 3], f32, tag="trip", name="trip")
    nc.vector.memset(trip[:, :, 0:1], float(BC))
    for ci, (k0, kc) in enumerate(KCH):
        nc.vector.bn_aggr(locmv[0:kc, ci, :], bst[0:kc, ci, :, :])
    nc.vector.tensor_copy(trip[:, :, 1:2], locmv[:, :, 0:1])
    nc.vector.tensor_scalar_mul(trip[:, :, 2:3], locmv[:, :, 1:2], float(BC))

    if l1_only:
        nc.vector.memset(outbuf32[:], 0.0)
        nc.sync.dma_start(
            io["out"].ap().rearrange("(q s) d -> q s d", q=128),
            outbuf32[:])
        return

    ag1_in = dram.tile([128, 9], f32, tag="ag1_in", name="ag1_in")
    ag1_out = dram.tile([ranks * 128, 9], f32, tag="ag1_out", name="ag1_out")
    nc.sync.dma_start(ag1_in[:], trip[:].rearrange("p a b -> p (a b)"))
    nc.gpsimd.collective_compute(
        "AllGather", ALU.bypass,
        replica_groups=[list(range(ranks))],
        ins=[ag1_in.opt()], outs=[ag1_out.opt()])

    allst1 = pp.tile([128, ranks, 3, 3], f32, tag="allst1", name="allst1")
    nc.sync.dma_start(
        allst1[:].rearrange("p r a b -> p r (a b)"),
        ag1_out.rearrange("(r p) c -> p r c", p=128))
    gst1 = pp.tile([128, 3, 2], f32, tag="gst1", name="gst1")
    for ci, (k0, kc) in enumerate(KCH):
        nc.vector.bn_aggr(gst1[0:kc, ci, :], allst1[0:kc, :, ci, :])

    # a1 = gamma1 * rsqrt(var + eps), computed for all 3 chunks at once
    # (rows beyond kc hold garbage that is never read by w2aT)
    a1 = pp.tile([128, 3], f32, tag="a1", name="a1")
    vtmp = pp.tile([128, 3], f32, tag="vtmp", name="vtmp")
    nc.vector.tensor_scalar_add(vtmp[:], gst1[:, :, 1], EPS)
    nc.vector.reciprocal(vtmp[:], vtmp[:])
    nc.scalar.activation(vtmp[:], vtmp[:], AF.Sqrt)
    nc.vector.tensor_mul(a1[:], vtmp[:], g1sb[:])

    w2aT = []
    for ci, (k0, kc) in enumerate(KCH):
        wa = pp.tile([128, O], f16, tag=f"w2aT{ci}", name=f"w2aT{ci}")
        nc.vector.tensor_scalar(
            wa[0:kc, :], w2bT[ci][0:kc, :], a1[0:kc, ci:ci + 1], None,
            op0=ALU.mult)
        w2aT.append(wa)

    # ---------------- layer 2 ----------------
    for g in range(NGRP):
        op_ = ps_o.tile([O, GW], f32, tag="hps", name="ops")
        for ci, (k0, kc) in enumerate(KCH):
            nc.tensor.matmul(
                op_[:], w2aT[ci][0:kc, :], hT[ci][0:kc, GW * g:GW * (g + 1)],
                start=(ci == 0), stop=(ci == 2))
        nc.scalar.copy(oT16[0:O, GW * g:GW * (g + 1)], op_[:])
        nc.vector.bn_stats(bst2[:, g, :], op_[:])

    # one batched xbar transpose [16, 64, 128] -> [128, 64, 16]; overlaps
    # the BN2 stats AllGather
    nc.sync.dma_start(outbuf16[:],
                      oT16[:].rearrange("p (s b) -> p s b", b=128),
                      transpose=True)

    # ---------------- BN2 stats exchange ----------------
    # ship (sum, sumsq) per feature; after the AG, one ones-matmul
    # broadcasts the gathered 160 floats (plus gamma2/beta2 staged at
    # startup) to all 128 partitions so the whole a2/b2 computation runs
    # full-width in the free dim - no 10-partition math, no transposes
    locmv2 = pp.tile([O, 2], f32, tag="locmv2", name="locmv2")
    sq2 = pp.tile([O, 2], f32, tag="sq2", name="sq2")
    nc.vector.bn_aggr(locmv2[:], bst2[:])
    nc.vector.tensor_mul(sq2[:, 1:2], locmv2[:, 0:1], locmv2[:, 0:1])
    nc.vector.tensor_add(sq2[:, 1:2], sq2[:, 1:2], locmv2[:, 1:2])
    nc.vector.tensor_scalar_mul(sq2[:, 1:2], sq2[:, 1:2], float(BC))
    nc.vector.tensor_scalar_mul(sq2[:, 0:1], locmv2[:, 0:1], float(BC))
    ag2_in = dram.tile([O, 2], f32, tag="ag2_in", name="ag2_in")
    ag2_out = dram.tile([ranks * O, 2], f32, tag="ag2_out", name="ag2_out")
    nc.sync.dma_start(ag2_in[:], sq2[:])
    nc.gpsimd.collective_compute(
        "AllGather", ALU.bypass,
        replica_groups=[list(range(ranks))],
        ins=[ag2_in.opt()], outs=[ag2_out.opt()])
    nc.sync.dma_start(stage[0:1, 0:20 * ranks],
                      ag2_out.rearrange("a b -> (a b)").unsqueeze(0))

    ones1 = pp.tile([1, 128], f32, tag="ones1", name="ones1")
    nc.vector.memset(ones1[:], 1.0)
    bc_ps = ps_w.tile([128, 20 * ranks + 2 * O], f32, tag="wps", name="bc_ps")
    nc.tensor.matmul(bc_ps[:], ones1[:], stage[:], start=True, stop=True)
    allbc = pp.tile([128, 20 * ranks + 2 * O], f32, tag="allbc", name="allbc")
    nc.vector.tensor_copy(allbc[:], bc_ps[:])

    # tree-reduce the 8 ranks' (sum, sumsq) pairs, then the affine consts
    nc.vector.tensor_add(allbc[:, 0:80], allbc[:, 0:80], allbc[:, 80:160])
    nc.vector.tensor_add(allbc[:, 0:40], allbc[:, 0:40], allbc[:, 40:80])
    nc.vector.tensor_add(allbc[:, 0:20], allbc[:, 0:20], allbc[:, 20:40])
    g20 = allbc[:, 0:20].rearrange("p (f c) -> p f c", c=2)
    a2bc = pp.tile([128, O], f32, tag="a2bc", name="a2bc")
    b2bc = pp.tile([128, O], f32, tag="b2bc", name="b2bc")
    mean2 = pp.tile([128, 2, O], f32, tag="mean2", name="mean2")
    nc.vector.tensor_scalar_mul(mean2[:, 0, :], g20[:, :, 0], inv_n)
    nc.vector.tensor_scalar_mul(mean2[:, 1, :], g20[:, :, 1], inv_n)
    nc.vector.tensor_mul(b2bc[:], mean2[:, 0, :], mean2[:, 0, :])
    nc.vector.tensor_sub(a2bc[:], mean2[:, 1, :], b2bc[:])
    nc.vector.tensor_scalar_add(a2bc[:], a2bc[:], EPS)
    nc.vector.reciprocal(a2bc[:], a2bc[:])
    nc.scalar.activation(a2bc[:], a2bc[:], AF.Sqrt)
    nc.vector.tensor_mul(a2bc[:], a2bc[:], allbc[:, 160:160 + O])
    nc.vector.tensor_mul(b2bc[:], mean2[:, 0, :], a2bc[:])
    nc.vector.tensor_sub(b2bc[:], allbc[:, 160 + O:160 + 2 * O], b2bc[:])

    # ---------------- final affine + store ----------------
    # halved so the first half's store overlaps the second half's affine
    outdst = io["out"].ap().rearrange("(q s) d -> q s d", q=128)
    hs2 = BC // 256
    for hh in range(2):
        sl = slice(hs2 * hh, hs2 * (hh + 1))
        nc.vector.tensor_mul(
            outbuf32[:, sl, :], outbuf16[:, sl, 0:O],
            a2bc[:].unsqueeze(1).broadcast_to([128, hs2, O]))
        nc.vector.tensor_add(
            outbuf32[:, sl, :], outbuf32[:, sl, :],
            b2bc[:].unsqueeze(1).broadcast_to([128, hs2, O]))
        nc.sync.dma_start(outdst[:, sl, :], outbuf32[:, sl, :])


def _build(debug=False, ranks=N_CORES, reps=1, l1_only=False):
    nc = bacc.Bacc("TRN2", target_bir_lowering=False, debug=False,
                   num_devices=ranks)

    io = {
        "x": nc.dram_tensor("x", [BC, D], f32, kind="ExternalInput"),
        "W1": nc.dram_tensor("W1", [H, D], f32, kind="ExternalInput"),
        "W2": nc.dram_tensor("W2", [O, H], f32, kind="ExternalInput"),
        "gamma1": nc.dram_tensor("gamma1", [H, 1], f32, kind="ExternalInput"),
        "gamma2": nc.dram_tensor("gamma2", [O, 1], f32, kind="ExternalInput"),
        "beta2": nc.dram_tensor("beta2", [O, 1], f32, kind="ExternalInput"),
        "out": nc.dram_tensor("out", [BC, O], f32, kind="ExternalOutput"),
    }
    if debug:
        io["h_dbg"] = nc.dram_tensor("h_dbg", [3, 128, NGRP * GW], f16,
                                     kind="ExternalOutput")

    with tile.TileContext(nc) as tc:
        with tc.tile_pool(name="persist", bufs=1) as pp, \
             tc.tile_pool(name="wtmp", bufs=1) as wtmp, \
             tc.tile_pool(name="xio", bufs=4) as xio, \
             tc.tile_pool(name="xTp", bufs=3) as xTp, \
             tc.tile_pool(name="scr", bufs=2) as scr, \
             tc.tile_pool(name="ps_h", bufs=3, space="PSUM") as ps_h, \
             tc.tile_pool(name="ps_t", bufs=4, space="PSUM") as ps_t, \
             tc.tile_pool(name="ps_w", bufs=1, space="PSUM") as ps_w, \
             tc.tile_pool(name="dram", bufs=1, space="DRAM") as dram:
            P = dict(pp=pp, wtmp=wtmp, xio=xio, xTp=xTp, scr=scr,
                     ps_h=ps_h, ps_t=ps_t, ps_w=ps_w, dram=dram)
            for _ in range(reps):
                _emit(nc, tc, io, P, ranks, debug, l1_only)

    nc.compile()
    return nc


_CACHE = {}


def get_nc(debug=False, ranks=N_CORES, reps=1, l1_only=False):
    key = (debug, ranks, reps, l1_only)
    if key not in _CACHE:
        _CACHE[key] = _build(debug, ranks, reps, l1_only)
    return _CACHE[key]


def make_in_maps(x, W1, gamma1, W2, gamma2, beta2, ranks=N_CORES):
    x = np.ascontiguousarray(np.asarray(x, dtype=np.float32))
    W1 = np.ascontiguousarray(np.asarray(W1, dtype=np.float32))
    W2 = np.ascontiguousarray(np.asarray(W2, dtype=np.float32))
    g1 = np.ascontiguousarray(np.asarray(gamma1, dtype=np.float32)).reshape(H, 1)
    g2 = np.ascontiguousarray(np.asarray(gamma2, dtype=np.float32)).reshape(O, 1)
    b2 = np.ascontiguousarray(np.asarray(beta2, dtype=np.float32)).reshape(O, 1)
    return [{
        "x": x[c * BC:(c + 1) * BC],
        "W1": W1, "W2": W2, "gamma1": g1, "gamma2": g2, "beta2": b2,
    } for c in range(ranks)]


def kernel(x, W1, gamma1, beta1, W2, gamma2, beta2):
    nc = get_nc()
    in_maps = make_in_maps(x, W1, gamma1, W2, gamma2, beta2)
    res = bass_utils.run_bass_kernel_spmd(
        nc, in_maps, core_ids=list(range(N_CORES)))
    return np.concatenate(
        [res.results[c]["out"] for c in range(N_CORES)], axis=0)

